# revision 1
# baseline (speedup 1.0000x reference)
"""Trainium2 Bass kernel for MiAttention (GQA + RoPE + causal attention).

Problem: B=1, S=4096, D=2048, H=16 q-heads, KVH=4 kv-heads, HD=128, fp32.
Sharding: tensor-parallel over heads across 8 cores. Core c computes q-heads
{2c, 2c+1} and kv-head c//2, produces a partial out-projection [S, D]; the 8
partials are summed on host (the "all-reduce").

Device-side layout strategy (per core):
  - hiddenT [D, S] bf16 is prepared on host; all projection matmuls contract
    over D on the partition axis, so no on-device transposes of activations.
  - qT [HD, S] and kT [HD, S] are produced directly in transposed layout
    (head-dim on partitions), which is what attention wants. RoPE is applied
    in this layout (rotate-half is a partition-slice swap).
  - v is produced as vT [HD, S] then PE-transposed to natural [S, HD] chunks
    (v is the stationary operand of the P@V matmul).
  - Attention runs in "scores-transposed" layout: ST[k, q] = k . q so that the
    post-softmax P tile (k on partitions) feeds P@V with no transpose.
    Softmax has no max-subtraction (scores are bounded ~ +-5 by construction),
    exp runs on the scalar engine straight out of PSUM with the 1/sqrt(HD)
    scale folded in. The denominator is a ones-vector matmul on the PE
    (partition-axis reduction), accumulated across k-tiles in PSUM.
  - Causal masking: k-tiles strictly below the diagonal need no mask; the
    diagonal k-tile gets a triangular mask multiply, fully-invalid q columns
    are zeroed.
  - out-projection consumes attn-outT [HD*2, S] as lhsT directly.
"""

import sys

sys.path.insert(0, "/opt/trn_rl_repo")

import numpy as np
import ml_dtypes
from contextlib import ExitStack

import concourse.bass as bass
from concourse import bacc
import concourse.mybir as mybir
import concourse.tile as tile
from concourse.masks import make_identity, make_upper_triangular

BF16 = mybir.dt.bfloat16
F32 = mybir.dt.float32

D = 2048
H = 16
KVH = 4
HD = 128
NCORES = 8
HPC = H // NCORES  # q heads per core = 2
ROPE_BASE = 10000.0
SCALE = 1.0 / float(np.sqrt(HD))
SC = 512  # seq chunk (psum free dim)
P = 128


def build_nc(S, reps=1, mode='full'):
    assert S % SC == 0
    NSC = S // SC  # seq chunks
    NKT = S // P  # k tiles
    DK = D // P  # contraction chunks over D

    nc = bacc.Bacc()
    hT = nc.dram_tensor("hT", [D, S], BF16, kind="ExternalInput")
    wqT = nc.dram_tensor("wqT", [D, HPC * HD], BF16, kind="ExternalInput")
    wkT = nc.dram_tensor("wkT", [D, HD], BF16, kind="ExternalInput")
    wvT = nc.dram_tensor("wvT", [D, HD], BF16, kind="ExternalInput")
    woT = nc.dram_tensor("woT", [HPC * HD, D], BF16, kind="ExternalInput")
    cosh = nc.dram_tensor("cosh", [HD // 2, S], F32, kind="ExternalInput")
    sinh = nc.dram_tensor("sinh", [HD // 2, S], F32, kind="ExternalInput")
    outp = nc.dram_tensor("outp", [S, D], BF16, kind="ExternalOutput")

    hT_r = hT.rearrange("(o p) s -> p o s", p=P)  # [128, DK, S]
    wq_r = wqT.rearrange("(o p) m -> p o m", p=P)  # [128, DK, 256]
    wk_r = wkT.rearrange("(o p) m -> p o m", p=P)
    wv_r = wvT.rearrange("(o p) m -> p o m", p=P)
    wo_r = woT.rearrange("(h p) n -> p h n", p=P)  # [128, HPC, D]
    out_r = outp.rearrange("(t p) d -> t p d", p=P)  # [NKT, 128, D]

    with tile.TileContext(nc) as tc, ExitStack() as ctx:
        consts = ctx.enter_context(tc.tile_pool(name="consts", bufs=1))
        persist = ctx.enter_context(tc.tile_pool(name="persist", bufs=1))

        # constants
        identity = consts.tile([P, P], BF16)
        make_identity(nc, identity)
        ones_col = consts.tile([P, 1], BF16)
        nc.vector.memset(ones_col, 1.0)
        trimask = consts.tile([P, P], BF16)
        make_upper_triangular(nc, trimask, val=1.0, diag=True)
        allones = consts.tile([P, P], F32)
        nc.vector.memset(allones, 1.0)
        # reciprocal rows, zero-padded to 128 partitions: partition 0 carries
        # 1/sum, the all-ones matmul broadcasts it to all 128 partitions.
        # One per head to avoid cross-iteration WAR serialization.
        rec_pad = []
        for h in range(HPC):
            rp_t = consts.tile([P, SC], F32, name=f"rec_pad_{h}")
            nc.vector.memset(rp_t, 0.0)
            rec_pad.append(rp_t)

        # resident weights
        wq_sb = consts.tile([P, DK, HPC * HD], BF16)
        nc.sync.dma_start(wq_sb, wq_r)
        wk_sb = consts.tile([P, DK, HD], BF16)
        nc.sync.dma_start(wk_sb, wk_r)
        wv_sb = consts.tile([P, DK, HD], BF16)
        nc.sync.dma_start(wv_sb, wv_r)
        wo_sb = consts.tile([P, HPC, D], BF16)
        nc.sync.dma_start(wo_sb, wo_r)
        cos_sb = consts.tile([HD // 2, S], F32)
        nc.sync.dma_start(cos_sb, cosh[:, :])
        sin_sb = consts.tile([HD // 2, S], F32)
        nc.sync.dma_start(sin_sb, sinh[:, :])

        # persistent activations
        qT_sb = persist.tile([P, HPC, S], BF16)  # rope'd q, transposed
        kT_sb = persist.tile([P, S], BF16)  # rope'd k, transposed
        v_sb = persist.tile([P, NKT, HD], BF16)  # v natural [k, hd] chunks
        aoT_sb = persist.tile([P, HPC, S], BF16)  # attention out, transposed

        HF = HD // 2  # 64
        _dummy = [None]  # py closure helper

        def rope(dst, src_ps, s0, s1):
            # dst[0:64]  = src[0:64]*cos - src[64:128]*sin
            # dst[64:128]= src[64:128]*cos + src[0:64]*sin
            # cos/sin halves are identical so only [64, S] tables are kept.
            # PSUM is staged through SBUF on the (here idle) scalar engine so
            # the six DVE multiplies run in fp32 2x SBUF mode.
            n = s1 - s0
            # two base-0 staging halves: SBUF-SBUF DVE ops require equal
            # base partitions across inputs
            s_lo = rope_tmp.tile([HF, n], F32, tag="rlo")
            s_hi = rope_tmp.tile([HF, n], F32, tag="rhi")
            nc.scalar.copy(s_lo, src_ps[0:HF, :])
            nc.scalar.copy(s_hi, src_ps[HF:P, :])
            t_a = rope_tmp.tile([HF, n], F32, tag="ra")
            t_b = rope_tmp.tile([HF, n], F32, tag="rb")
            cs = cos_sb[:, s0:s1]
            sn = sin_sb[:, s0:s1]
            nc.vector.tensor_tensor(t_a, s_hi, sn, mybir.AluOpType.mult)
            nc.vector.tensor_tensor(t_b, s_lo, cs, mybir.AluOpType.mult)
            nc.vector.tensor_tensor(dst[0:HF, s0:s1], t_b, t_a, mybir.AluOpType.subtract)
            nc.vector.tensor_tensor(t_a, s_lo, sn, mybir.AluOpType.mult)
            nc.vector.tensor_tensor(t_b, s_hi, cs, mybir.AluOpType.mult)
            nc.vector.tensor_tensor(dst[HF:P, s0:s1], t_b, t_a, mybir.AluOpType.add)

        for _rep in range(reps):
            if mode == 'attn' and True:
                nc.vector.memset(qT_sb, 0.0)
                nc.vector.memset(kT_sb, 0.0)
                nc.vector.memset(v_sb, 0.0)
            # ---------------- phase 1: projections + rope + v transpose ----------
            with (
                tc.tile_pool(name="hpool", bufs=2) as hpool,
                tc.tile_pool(name="rope_tmp", bufs=4) as rope_tmp,
                tc.tile_pool(name="vt_tmp", bufs=2) as vt_tmp,
                tc.tile_pool(name="pp", bufs=3, space="PSUM") as pp,
                tc.tile_pool(name="tp", bufs=2, space="PSUM") as tp,
            ):
                for sc in (range(NSC) if mode != 'attn' else []):
                    s0, s1 = sc * SC, (sc + 1) * SC
                    h_tile = hpool.tile([P, DK, SC], BF16, tag="h")
                    nc.sync.dma_start(h_tile, hT_r[:, :, s0:s1])

                    # q projections (2 heads)
                    for m in range(HPC):
                        q_ps = pp.tile([P, SC], F32, tag="proj")
                        for k in range(DK):
                            nc.tensor.matmul(
                                q_ps,
                                wq_sb[:, k, m * HD : (m + 1) * HD],
                                h_tile[:, k, :],
                                start=(k == 0),
                                stop=(k == DK - 1),
                            )
                        rope(qT_sb[:, m], q_ps, s0, s1)

                    # k projection
                    k_ps = pp.tile([P, SC], F32, tag="proj")
                    for k in range(DK):
                        nc.tensor.matmul(
                            k_ps, wk_sb[:, k, :], h_tile[:, k, :],
                            start=(k == 0), stop=(k == DK - 1),
                        )
                    rope(kT_sb, k_ps, s0, s1)

                    # v projection (transposed), then PE-transpose to natural
                    v_ps = pp.tile([P, SC], F32, tag="proj")
                    for k in range(DK):
                        nc.tensor.matmul(
                            v_ps, wv_sb[:, k, :], h_tile[:, k, :],
                            start=(k == 0), stop=(k == DK - 1),
                        )
                    vt_sb = vt_tmp.tile([P, SC], BF16, tag="vt")
                    nc.scalar.copy(vt_sb, v_ps)
                    for j in range(SC // P):
                        t_ps = tp.tile([P, P], BF16, tag="tps")
                        nc.tensor.transpose(t_ps, vt_sb[:, j * P : (j + 1) * P], identity)
                        nc.vector.tensor_copy(v_sb[:, sc * (SC // P) + j, :], t_ps)

            # ------- phase 2: attention + fused out-projection per q-chunk -------
            # Heads are interleaved in the inner k-loop: shares LDWEIGHTS
            # (kT/ones/v tiles are lhsT for both heads) and gives the PE two
            # independent dependency chains to hide the exp (ACT) latency.
            with (
                tc.tile_pool(name="ppool", bufs=6) as ppool,
                tc.tile_pool(name="nrm", bufs=2) as nrm,
                tc.tile_pool(name="orow", bufs=2) as orow,
                tc.tile_pool(name="st", bufs=2, space="PSUM") as st,
                tc.tile_pool(name="opsum", bufs=2, space="PSUM") as opsum,
                tc.tile_pool(name="ssum", bufs=1, space="PSUM") as ssum,
                tc.tile_pool(name="misc", bufs=1, space="PSUM") as misc,
            ):
                SKEW = 2  # scoresT pairs issued this many k-tiles ahead

                def issue_scores(qc, kk):
                    # scoresT matmuls for both heads into one bf16 PSUM tile
                    # (1 bank), then a single exp (N=1024 amortizes the ACT
                    # fixed overhead) and a single causal-mask select.
                    q0, q1 = qc * SC, (qc + 1) * SC
                    s_ps = st.tile([P, HPC, SC], F32, tag="st")
                    for hh in range(HPC):
                        nc.tensor.matmul(
                            s_ps[:, hh, :],
                            kT_sb[:, kk * P : (kk + 1) * P],
                            qT_sb[:, hh, q0:q1],
                            start=True, stop=True,
                        )
                    pt = ppool.tile([P, HPC, SC], BF16, tag="p")
                    nc.scalar.activation(
                        pt, s_ps, mybir.ActivationFunctionType.Exp, scale=SCALE
                    )
                    if kk >= qc * (SC // P):
                        # tile straddles the causal diagonal: one affine_select
                        # keeps q >= k, zeroes the rest (idle GPSIMD)
                        nc.gpsimd.affine_select(
                            out=pt,
                            in_=pt,
                            compare_op=mybir.AluOpType.is_ge,
                            fill=0.0,
                            base=qc * SC - kk * P,
                            pattern=[[0, HPC], [1, SC]],
                            channel_multiplier=-1,
                        )
                    return pt

                # flat software pipeline across all (qc, kk) pairs so score
                # issue runs SKEW ahead even across q-chunk boundaries
                sched = ([] if mode == 'proj' else
                         [(qc, kk) for qc in range(NSC)
                          for kk in range((qc + 1) * (SC // P))])
                pending = {}
                issued = 0
                o_ps = {}
                s_sum = {}
                for i, (qc, kk) in enumerate(sched):
                    while issued < min(i + 1 + SKEW, len(sched)):
                        pending[sched[issued]] = issue_scores(*sched[issued])
                        issued += 1
                    kmax = (qc + 1) * (SC // P)
                    q0, q1 = qc * SC, (qc + 1) * SC
                    if kk == 0:
                        o_ps[qc] = [opsum.tile([P, SC], F32, tag="o",
                                               name=f"o_{qc}_{h}")
                                    for h in range(HPC)]
                        # both heads' denominators share one PSUM bank
                        # (matmul outputs must start at partition 0/32/64)
                        s_sum_t = ssum.tile([33, SC], F32, tag="s", name=f"s_{qc}")
                        s_sum[qc] = [s_sum_t[0:1, :], s_sum_t[32:33, :]]
                    p_sb = pending.pop((qc, kk))
                    for hh in range(HPC):
                        nc.tensor.matmul(
                            s_sum[qc][hh], ones_col, p_sb[:, hh, :],
                            start=(kk == 0), stop=(kk == kmax - 1),
                        )
                    for hh in range(HPC):
                        nc.tensor.matmul(
                            o_ps[qc][hh], v_sb[:, kk, :], p_sb[:, hh, :],
                            start=(kk == 0), stop=(kk == kmax - 1),
                        )
                    if kk != kmax - 1:
                        continue
                    # ---- end of q-chunk: normalize + fused out-projection ----
                    for hh in range(HPC):
                        nc.vector.reciprocal(rec_pad[hh][0:1, :], s_sum[qc][hh])
                        bc_ps = misc.tile([P, SC], F32, tag="m")
                        nc.tensor.matmul(bc_ps, allones, rec_pad[hh], start=True, stop=True)
                        bc_sb = nrm.tile([P, SC], F32, tag="bc")
                        nc.vector.tensor_copy(bc_sb, bc_ps)
                        nc.vector.tensor_tensor(
                            aoT_sb[:, hh, q0:q1], o_ps[qc][hh], bc_sb,
                            mybir.AluOpType.mult
                        )
                    del o_ps[qc], s_sum[qc]
                    for t in range(qc * (SC // P), (qc + 1) * (SC // P)):
                        row_sb = orow.tile([P, D], BF16, tag="row")
                        for n in range(D // SC):
                            o2_ps = misc.tile([P, SC], F32, tag="m")
                            for hh in range(HPC):
                                nc.tensor.matmul(
                                    o2_ps,
                                    aoT_sb[:, hh, t * P : (t + 1) * P],
                                    wo_sb[:, hh, n * SC : (n + 1) * SC],
                                    start=(hh == 0), stop=(hh == HPC - 1),
                                )
                            if n % 2 == 0:
                                nc.vector.tensor_copy(row_sb[:, n * SC : (n + 1) * SC], o2_ps)
                            else:
                                nc.scalar.copy(row_sb[:, n * SC : (n + 1) * SC], o2_ps)
                        nc.gpsimd.dma_start(out_r[t], row_sb)

    nc.finalize()
    return nc


def host_prep(hidden_states, Wq, Wk, Wv, Wo, position_ids):
    """Shard + pre-transpose + cast inputs for the 8 cores."""
    bf16 = ml_dtypes.bfloat16
    S = hidden_states.shape[1]
    h = np.asarray(hidden_states, dtype=np.float32).reshape(S, D)
    hT = np.ascontiguousarray(h.T.astype(bf16))  # [D, S]

    pos = np.asarray(position_ids).reshape(-1)[:S].astype(np.float32)
    inv_freq = (1.0 / (ROPE_BASE ** (np.arange(0, HD, 2, dtype=np.float32) / HD))).astype(np.float32)
    freqs = pos[None, :] * inv_freq[:, None]  # [64, S]
    cosh = np.cos(freqs).astype(np.float32)
    sinh = np.sin(freqs).astype(np.float32)

    Wq = np.asarray(Wq, dtype=np.float32)
    Wk = np.asarray(Wk, dtype=np.float32)
    Wv = np.asarray(Wv, dtype=np.float32)
    Wo = np.asarray(Wo, dtype=np.float32)

    in_maps = []
    for c in range(NCORES):
        qlo, qhi = 2 * c * HD, (2 * c + 2) * HD
        g = c // 2
        in_maps.append({
            "hT": hT,
            "wqT": np.ascontiguousarray(Wq[qlo:qhi, :].T.astype(bf16)),
            "wkT": np.ascontiguousarray(Wk[g * HD : (g + 1) * HD, :].T.astype(bf16)),
            "wvT": np.ascontiguousarray(Wv[g * HD : (g + 1) * HD, :].T.astype(bf16)),
            "woT": np.ascontiguousarray(Wo[:, qlo:qhi].T.astype(bf16)),
            "cosh": cosh,
            "sinh": sinh,
        })
    return in_maps


_NC_CACHE = {}


def _get_nc(S):
    if S not in _NC_CACHE:
        _NC_CACHE[S] = build_nc(S)
    return _NC_CACHE[S]


def kernel(hidden_states, Wq, Wk, Wv, Wo, position_ids):
    from concourse.bass_utils import run_bass_kernel_spmd

    hidden_states = np.asarray(hidden_states)
    B, S, _ = hidden_states.shape
    nc = _get_nc(S)
    in_maps = host_prep(hidden_states, Wq, Wk, Wv, Wo, position_ids)
    res = run_bass_kernel_spmd(nc, in_maps, list(range(NCORES)))
    partials = [np.asarray(res.results[i]["outp"], dtype=np.float32) for i in range(NCORES)]
    out = np.sum(np.stack(partials, axis=0), axis=0, dtype=np.float32)
    return out.reshape(B, S, D).astype(np.float32)

